# revision 61
# baseline (speedup 1.0000x reference)
"""Biquad peaking-EQ IIR filter on 8 Trainium2 NeuronCores.

Math: the reference applies a 2nd-order IIR (biquad) along time for each of
the 64 independent signals (32 batch x 2 channels, T=524288).  The filter's
poles have magnitude sqrt(a2) ~ 0.919, so the impulse response decays below
~2e-5 after 128 samples.  We compute the zero-state response as a
truncated-FIR convolution (taps 0..g+128 per sample, identical truncation to
the previous kernel, L2 err ~2e-6):

    y[128B + g] = sum_j x[128B + j] T0[j, g] + sum_j x[128(B-1) + j] T1[j, g]

with T0[j,g] = h[g-j] (g>=j), T1[j,g] = h[128+g-j].

Layout trick: tiles are loaded BLOCK-MAJOR straight from DRAM: an SBUF tile
X2[B, j] = x[16384t + 128B + j] is a plain [128, 128-col] strided DMA view
(512-byte contiguous lines), so both the input and the output tiles are
DMA-native and the only on-chip transposes are the PE pass-through
transposes X2 -> X' (X'[j, B]).  X' (cast to bf16 during PSUM evacuation)
is then the MATMUL STATIONARY operand:

    Y2[B, g] = X'[:, 1+128t+B].T @ W0  +  X'[:, 128t+B].T @ W1

where the one-block lookback of the T1 term is just a one-column shift of
the stationary SBUF slice (a zeroed halo column handles the signal start).
The output lands directly in the DMA-native [B, g] orientation - no output
transpose, no strided on-chip copies.

Per core (8 signals): PE = 256 fp32 transposes (2 cyc/row) + 512 bf16
matmuls (1 cyc/row) ~= 131k cycles ~= 55 us; DVE evacuates X' (64x
[128,512] casts), ACT evacuates Y2 (64x [128,512] copies); DMA moves
2 x 16.8 MB at ~360-400 GB/s aggregate => the kernel is DMA-bound.

Sharding: pure data parallel - 64 signals / 8 cores = 8 signals per core.

Scheduling note: every TPB 64-byte instruction has a single semaphore-wait
slot, but Tile's slot-release deps routinely put 2+ waits on one
instruction (walrus then fails with "Too many sync wait commands").
_strip_redundant_waits post-processes the scheduled BIR: it computes
transitive completion guarantees (engine queues are in-order FIFO; an
instruction completes only after its waits held; a semaphore's v-th update
implies its earlier ones) and (a) drops waits provably implied by another
wait on the same instruction, (b) splits any remaining multi-wait set into
single-wait NoOps ahead of the instruction on the same queue.  The patched
BIR is returned via an instance-level to_json_bytes override that
bass2jax's lowering picks up.
"""

import math

import numpy as np

SAMPLE_RATE = 44100.0

# Problem geometry (hardcoded per harness contract).
B_FULL, C_FULL, T_FULL = 32, 2, 524288
N_CORES = 8
SIGS_PER_CORE = (B_FULL * C_FULL) // N_CORES  # 8
L = 128            # block size == PE array dim
NBLK = T_FULL // L          # 4096 blocks per signal
NTILE = NBLK // L           # 32 [128 x 128] tiles per signal
NQ = 4                      # DMA quarters per signal
TPQ = NTILE // NQ           # 8 tiles per quarter


def _filter_coeffs(center_freq: float, q: float, gain: float):
    """torchaudio equalizer_biquad coefficients, normalized by a0 (float64)."""
    g = min(max(gain, 0.1), 10.0)
    w0 = 2.0 * math.pi * center_freq / SAMPLE_RATE
    A = math.exp(g / 40.0 * math.log(10.0))
    alpha = math.sin(w0) / (2.0 * q)
    b0 = 1.0 + alpha * A
    b1 = -2.0 * math.cos(w0)
    b2 = 1.0 - alpha * A
    a0 = 1.0 + alpha / A
    a1 = b1
    a2 = 1.0 - alpha / A
    return b0 / a0, b1 / a0, b2 / a0, a1 / a0, a2 / a0


def _impulse_response(center_freq: float, q: float, gain: float, n: int = 256):
    b0, b1, b2, a1, a2 = _filter_coeffs(center_freq, q, gain)
    h = np.zeros(n, dtype=np.float64)
    x1 = x2 = y1 = y2 = 0.0
    for i in range(n):
        xn = 1.0 if i == 0 else 0.0
        yn = b0 * xn + b1 * x1 + b2 * x2 - a1 * y1 - a2 * y2
        x2, x1 = x1, xn
        y2, y1 = y1, yn
        h[i] = yn
    return h


def _w_matrix(h: np.ndarray):
    """Moving operand W [128, 256] (bf16): cols 0..127 = T0[j,g] = h[g-j]
    (g>=j else 0), cols 128..255 = T1[j,g] = h[128+g-j]."""
    import ml_dtypes

    j = np.arange(L)[:, None]
    g = np.arange(L)[None, :]
    d0 = g - j
    t0 = np.where(d0 >= 0, h[np.clip(d0, 0, len(h) - 1)], 0.0)
    d1 = 128 + g - j  # in [1, 255]
    t1 = h[d1]
    w = np.concatenate([t0, t1], axis=1)
    return w.astype(ml_dtypes.bfloat16)


_NC_CACHE = {}


def _build_nc(n_sigs: int = SIGS_PER_CORE):
    """Build the per-core Bass program (same NEFF on all cores)."""
    import concourse.bass as bass
    import concourse.mybir as mybir
    import concourse.tile as tile
    from concourse.masks import make_identity

    f32 = mybir.dt.float32
    bf16 = mybir.dt.bfloat16
    nc = bass.Bass("TRN2")

    x = nc.dram_tensor("x", [n_sigs, T_FULL], bf16, kind="ExternalInput")
    w = nc.dram_tensor("w", [L, 2 * L], bf16, kind="ExternalInput")
    y = nc.dram_tensor("y", [n_sigs, T_FULL], bf16, kind="ExternalOutput")

    G = 8                 # blocks per output partition line (2 KiB bf16)
    NSUP = NBLK // (G * L)  # 4 supertiles per signal
    # X' source view: (s, B, j) = x[s, 128B + j] -- contiguous 2D per
    # signal, fed to the DMA transpose XBAR ([16, 128] source tiles = 4 KiB
    # sequential DRAM reads).  XBARs all stay on the Sync queue: two
    # concurrently-running XBAR transfers (e.g. from two queues) corrupt
    # each other's output.
    x_r = x[:].rearrange("s (B j) -> s B j", j=L)
    # Paired XBAR source: two DRAM-contiguous signals per transpose, halving
    # the number of (serially dispatched) XBAR instructions on the Sync
    # queue: (c, 8192, 128) covering signals 2c and 2c+1.
    x_p = x[:].rearrange("(c two) (B j) -> c (two B) j", two=2, j=L)
    # Flat all-signal XBAR source for variable-size signal groups.
    x_all = x[:].rearrange("s (B j) -> (s B) j", j=L)
    # Natural G-grouped input view for the PE-transpose signals: partition p
    # line = 2 KiB contiguous per supertile:
    # (s, p, tp, u, j) = x[s, 131072*tp + 1024*p + 128*u + j]
    x_g = x[:].rearrange(
        "s (tp p u j) -> s p tp (u j)", tp=NSUP, p=L, u=G, j=L
    )
    # G-grouped out view: partition m holds blocks {8m+u}, so each partition
    # line is (u, g) = 1024 samples = 2 KiB contiguous DRAM:
    # (s, m, tp, 1024-chunk) = y[s, 131072*tp + 1024*m + chunk]
    y_r = y[:].rearrange("s (tp m ug) -> s m tp ug", tp=NSUP, m=L, ug=G * L)

    with tile.TileContext(nc) as tc:
        with (
            tc.tile_pool(name="consts", bufs=1) as consts,
            tc.tile_pool(name="xp", bufs=3) as xp_pool,
            tc.tile_pool(name="xps", bufs=2) as xps_pool,
            tc.tile_pool(name="xg", bufs=2) as xg_pool,
            tc.tile_pool(name="xt", bufs=6) as xt_pool,
            tc.tile_pool(name="pad", bufs=8) as pad_pool,
            tc.tile_pool(name="yout", bufs=3) as yout_pool,
            tc.tile_pool(name="m_ps", bufs=4, space="PSUM") as m_ps,
            tc.tile_pool(name="t_ps", bufs=3, space="PSUM") as t_ps,
        ):
            w_raw = consts.tile([L, 2 * L], bf16)
            nc.sync.dma_start(w_raw[:], w[:])
            w_sb = consts.tile([L, 2 * L], bf16)
            nc.vector.tensor_copy(w_sb[:], w_raw[:])
            ident = consts.tile([L, L], bf16)
            make_identity(nc, ident[:])

            # Signals on the PE-transpose input path (plain 2 KiB-line DMA +
            # tensor-engine transposes instead of the XBAR).  Measured on
            # hardware: concurrent plain-DMA traffic degrades XBAR packet
            # throughput ~1.5x and its dispatch cost ~1.7x, so the hybrid is
            # a net loss -- keep every signal on the XBAR path.
            PE_SIGS = set()

            # Emit all XBAR loads first: with 8 resident X' tiles they are
            # consecutive on the Sync queue (back-to-back dispatch, input
            # stream never starves) and they claim hardware DMA semaphores
            # before any out-DMA, so the 8-sem round-robin never makes an
            # input wait on an output completion.
            xbar_tiles = {}
            if not PE_SIGS and n_sigs == 8:
                # Group XBARs 3+3+2: fewer serially-dispatched instructions
                # than pairs, and the smallest transfer lands last so the
                # final signals' matmuls (the kernel tail) start earlier.
                for s0, ng in ((0, 3), (3, 3), (6, 2)):
                    xpg = xp_pool.tile([L, ng * NBLK], bf16)
                    nc.sync.dma_start_transpose(
                        xpg[:], x_all[s0 * NBLK : (s0 + ng) * NBLK]
                    )
                    for i in range(ng):
                        xbar_tiles[s0 + i] = xpg[:, i * NBLK : (i + 1) * NBLK]
            else:
                for s in range(n_sigs):
                    if s not in PE_SIGS:
                        xp = xps_pool.tile([L, NBLK], bf16)
                        nc.sync.dma_start_transpose(xp[:], x_r[s])
                        xbar_tiles[s] = xp[:]

            for s in range(n_sigs):
                y2s = yout_pool.tile([L, NBLK], bf16)
                if s in PE_SIGS:
                    xg = xg_pool.tile([L, NBLK], bf16)
                    nc.scalar.dma_start(
                        xg[:].rearrange("p (tp c) -> p tp c", tp=NSUP),
                        x_g[s],
                    )
                    prev_xt = None
                    for tp in range(NSUP):
                        # X' supertile: col 128u + p <-> block 8p + u.
                        xt = xt_pool.tile([L, G * L], bf16)
                        for g2 in range(2):
                            tps = t_ps.tile([L, 512], bf16, tag="t")
                            for kk in range(4):
                                u = 4 * g2 + kk
                                nc.tensor.transpose(
                                    tps[:, L * kk : L * (kk + 1)],
                                    xg[:, 1024 * tp + L * u : 1024 * tp + L * (u + 1)],
                                    ident[:],
                                )
                            dst = xt[:, 512 * g2 : 512 * (g2 + 1)]
                            if g2 == 0:
                                nc.scalar.copy(dst, tps[:])
                            else:
                                nc.vector.tensor_copy(dst, tps[:])
                        # Pad: col m = block (1024*tp + 8m - 1).
                        pad = pad_pool.tile([L, L], bf16)
                        if tp == 0:
                            nc.vector.memset(pad[:, 0:1], 0.0)
                        else:
                            nc.vector.tensor_copy(
                                pad[:, 0:1], prev_xt[:, G * L - 1 : G * L]
                            )
                        nc.vector.tensor_copy(
                            pad[:, 1:L], xt[:, 7 * L : 7 * L + L - 1]
                        )
                        for half in range(2):
                            ps = m_ps.tile([L, 512], f32, tag="m")
                            for k in range(4):
                                u = 4 * half + k
                                nc.tensor.matmul(
                                    ps[:, L * k : L * (k + 1)],
                                    xt[:, L * u : L * (u + 1)],
                                    w_sb[:, 0:L],
                                    start=True, stop=False,
                                )
                                t1_stat = (
                                    pad[:] if u == 0
                                    else xt[:, L * (u - 1) : L * u]
                                )
                                nc.tensor.matmul(
                                    ps[:, L * k : L * (k + 1)],
                                    t1_stat,
                                    w_sb[:, L : 2 * L],
                                    start=False, stop=True,
                                )
                            base = 1024 * tp + 512 * half
                            dst = y2s[:, base : base + 512]
                            if half == 0:
                                nc.scalar.copy(dst, ps[:])
                            else:
                                nc.vector.tensor_copy(dst, ps[:])
                        prev_xt = xt
                else:
                    # X' via the DMA transpose XBAR (loaded above).  The dst
                    # starts at the tile base (any offset dst produces wrong
                    # XBAR output), so "block -1" halos live in pad tiles.
                    xp = xbar_tiles[s]
                    # Stationary views: (j, tp, u, k) = xp[j, 1024*tp+8k+u]
                    xpv = xp.rearrange(
                        "j (tp k u) -> j tp u k", tp=NSUP, k=L, u=G
                    )
                    for tp in range(NSUP):
                        # Pad tile for the u=0 lookback: col m holds block
                        # (1024*tp + 8m - 1): col 0 = previous supertile's
                        # last block (zero at signal start), cols 1.. = u=7
                        # stride-8 slice shifted by one k.
                        pad = pad_pool.tile([L, L], bf16)
                        if tp == 0:
                            nc.vector.memset(pad[:, 0:1], 0.0)
                        else:
                            nc.vector.tensor_copy(
                                pad[:, 0:1], xp[:, 1024 * tp - 1 : 1024 * tp]
                            )
                        nc.vector.tensor_copy(
                            pad[:, 1:L], xpv[:, tp, G - 1][:, 0 : L - 1]
                        )
                        for half in range(2):  # psum groups of 4 u's
                            ps = m_ps.tile([L, 512], f32, tag="m")
                            for k in range(4):
                                u = 4 * half + k
                                nc.tensor.matmul(
                                    ps[:, L * k : L * (k + 1)],
                                    xpv[:, tp, u],
                                    w_sb[:, 0:L],
                                    start=True, stop=False,
                                )
                                t1_stat = (
                                    pad[:] if u == 0 else xpv[:, tp, u - 1]
                                )
                                nc.tensor.matmul(
                                    ps[:, L * k : L * (k + 1)],
                                    t1_stat,
                                    w_sb[:, L : 2 * L],
                                    start=False, stop=True,
                                )
                            # Evac casts PSUM fp32 -> bf16; split DVE/ACT.
                            base = 1024 * tp + 512 * half
                            dst = y2s[:, base : base + 512]
                            if half == 0:
                                nc.scalar.copy(dst, ps[:])
                            else:
                                nc.vector.tensor_copy(dst, ps[:])
                # One 2KiB-line out-DMA per signal.  Alternate the Scalar and
                # Sync queues: each queue's ring drains one DMA at a time, so
                # two queues double the output stream rate (the Sync queue is
                # idle once the XBARs are dispatched).  Plain-DMA concurrency
                # across queues is safe -- only XBAR-vs-XBAR is not.
                eng = nc.scalar if s % 2 == 0 else nc.sync
                eng.dma_start(
                    y_r[s], y2s[:].rearrange("m (tp ug) -> m tp ug", tp=NSUP)
                )

    return nc


def _strip_redundant_waits(bir_bytes: bytes) -> bytes:
    """PE Matmult/Ldweights lower to TPB instructions with a single
    semaphore-wait slot, but Tile's slot-release deps put 2 waits (old-writer
    PE completion + old-reader DVE completion) on the first toucher of every
    reused PSUM slot.  The PE wait is transitively implied: the DVE evac copy
    whose completion the instruction also waits on had itself waited on those
    PE completions.  Prove the implication with a completion-guarantee
    dataflow (rules: an instruction completes only after its waits hold; TPB
    engine queues are in-order FIFO; a semaphore's v-th update implies its
    earlier updates) and drop provably-redundant waits; raise if a >1-wait
    matmul can't be reduced."""
    import json

    bir = json.loads(bir_bytes)
    insts = []
    containers = []  # (list, index) for each inst, for NoOp insertion

    def walk(block):
        lst = block.get("instructions", [])
        for idx, i in enumerate(lst):
            insts.append(i)
            containers.append((lst, idx))
        for sub in block.get("blocks", []):
            walk(sub)

    for b in bir["functions"][0]["blocks"]:
        walk(b)

    # Per-sem update timeline: list of (cumulative_value, inst_idx).
    timelines = {}
    for k, i in enumerate(insts):
        for u in i.get("sync_info", {}).get("on_update", []) or []:
            if u.get("sync_type") != "semaphore":
                continue
            tl = timelines.setdefault(u["ant_name"], [])
            prev = tl[-1][0] if tl else 0
            tl.append((prev + int(u.get("update_value", 1)), k))

    def producer(sem, val):
        """Index of the instruction whose update first brings sem >= val."""
        tl = timelines.get(sem)
        if not tl:
            return None
        import bisect
        pos = bisect.bisect_left(tl, (val, -1))
        if pos == len(tl):
            return None
        return tl[pos][1]

    IN_ORDER_ENGINES = {"PE", "DVE", "Activation", "Pool", "SP"}
    # complete out-of-band on DMA queues
    NOT_IN_ORDER_OPCODES = {"DMACopy", "DmaTransposeAnt"}

    # guarantees[k]: sem -> max value known to hold when inst k completes.
    guarantees = [dict() for _ in insts]
    prev_by_engine = {}
    preds = []  # per-inst: (same-engine pred, own waits, own updates)
    for k, i in enumerate(insts):
        eng = i.get("engine")
        in_order = eng in IN_ORDER_ENGINES and i.get("opcode") not in NOT_IN_ORDER_OPCODES
        pred = prev_by_engine.get(eng) if in_order else None
        preds.append(pred)
        if in_order:
            prev_by_engine[eng] = k

    def merge(dst, src):
        changed = False
        for s, v in src.items():
            if dst.get(s, 0) < v:
                dst[s] = v
                changed = True
        return changed

    for _pass in range(3):
        changed = False
        for k, i in enumerate(insts):
            g = guarantees[k]
            si = i.get("sync_info", {})
            for w in si.get("on_wait", []) or []:
                if w.get("sync_type") != "semaphore":
                    continue
                v = int(w["wait_value"])
                if g.get(w["ant_name"], 0) < v:
                    g[w["ant_name"]] = v
                    changed = True
                p = producer(w["ant_name"], v)
                if p is not None:
                    changed |= merge(g, guarantees[p])
            # A DMA dispatch may launch before earlier same-queue engine
            # instructions complete, so only non-DMA instructions inherit
            # their queue predecessor's guarantees.
            if preds[k] is not None and i.get("opcode") not in NOT_IN_ORDER_OPCODES:
                changed |= merge(g, guarantees[preds[k]])
        # Own updates fire at completion; same-sem update chains are FIFO
        # (engine queue or DMA queue), so the v-th updater inherits the
        # (v-1)-th updater's guarantees.
        for sem, tl in timelines.items():
            prev_idx = None
            for cum, k in tl:
                if guarantees[k].get(sem, 0) < cum:
                    guarantees[k][sem] = cum
                    changed = True
                if prev_idx is not None:
                    changed |= merge(guarantees[k], guarantees[prev_idx])
                prev_idx = k
        if not changed:
            break

    STRIP_OPCODES = {
        "Matmult", "Ldweights", "TensorCopy", "Memset", "DMACopy",
        "DmaTransposeAnt", "Activation", "TensorScalarAffineSelect",
        "TensorTensor", "TensorScalarPtr", "TensorReduce", "Drain", "NoOp",
    }
    stripped = 0
    inserts = []  # (list, index, [noop dicts])
    for k, i in enumerate(insts):
        if i.get("opcode") not in STRIP_OPCODES:
            continue
        si = i.get("sync_info", {})
        waits = si.get("on_wait", []) or []
        if len(waits) <= 1:
            continue
        # Drop every wait implied by another (not-yet-dropped) wait's
        # producer guarantee.  DMA-class instructions complete out-of-band
        # and their cross-queue ordering is subtle: never REDUCE their
        # waits, only split them onto queue-holding NoOps (always sound).
        kept = list(waits)
        if i.get("opcode") not in NOT_IN_ORDER_OPCODES:
            changed = True
            while changed:
                changed = False
                for w in list(kept):
                    if len(kept) == 1:
                        break
                    for w2 in kept:
                        if w2 is w:
                            continue
                        p = producer(w2["ant_name"], int(w2["wait_value"]))
                        if p is not None and guarantees[p].get(w["ant_name"], 0) >= int(
                            w["wait_value"]
                        ):
                            kept.remove(w)
                            changed = True
                            break
        stripped += len(waits) - len(kept)
        si["on_wait"] = [kept[-1]]
        if len(kept) > 1:
            # Split remaining waits onto single-wait NoOps ahead of the
            # instruction on the same engine queue.
            lst, idx = containers[k]
            noops = [
                {
                    "debug": i.get("debug", 0),
                    "engine": i.get("engine"),
                    "ins": [],
                    "name": f"{i['name']}-w{j}",
                    "opcode": "NoOp",
                    "outs": [],
                    "sync_info": {"on_wait": [w], "on_update": []},
                }
                for j, w in enumerate(kept[:-1])
            ]
            inserts.append((lst, idx, noops))

    # Apply insertions (descending index per list keeps positions valid).
    from collections import defaultdict
    by_list = defaultdict(list)
    for lst, idx, noops in inserts:
        by_list[id(lst)].append((lst, idx, noops))
    for entries in by_list.values():
        for lst, idx, noops in sorted(entries, key=lambda e: -e[1]):
            lst[idx:idx] = noops

    out = json.dumps(bir).encode()
    return out


def audit_waits(bir_bytes):
    """Flag Matmult/Ldweights instructions with more than the single
    hardware wait slot."""
    import json

    bir = json.loads(bir_bytes)
    checked = {
        "Matmult", "Ldweights", "TensorCopy", "Memset", "DMACopy",
        "DmaTransposeAnt", "Activation", "TensorScalarAffineSelect",
        "TensorTensor", "TensorScalarPtr", "TensorReduce",
    }
    bad = []
    def walk(block):
        for i in block.get("instructions", []):
            if i.get("opcode") not in checked:
                continue
            w = i.get("sync_info", {}).get("on_wait", [])
            if len(w) > 1:
                bad.append((i["name"], i.get("opcode"), i.get("engine"),
                            [(x["ant_name"], x["wait_value"]) for x in w]))
        for sub in block.get("blocks", []):
            walk(sub)
    for b in bir["functions"][0]["blocks"]:
        walk(b)
    return bad


def _get_nc(n_sigs: int = SIGS_PER_CORE):
    if n_sigs not in _NC_CACHE:
        nc = _build_nc(n_sigs)
        patched = _strip_redundant_waits(type(nc).to_json_bytes(nc))
        bad = audit_waits(patched)
        if bad:
            raise RuntimeError(f"multi-wait PE instructions remain: {bad[:5]}")
        nc.to_json_bytes = lambda: patched
        _NC_CACHE[n_sigs] = nc
    return _NC_CACHE[n_sigs]


def run_spmd(x64: np.ndarray, w: np.ndarray, trace: bool = False):
    """x64: [64, T] float32, w: [128, 256] bf16 -> [64, T] float32.

    x/y transit DRAM as bf16 (host casts): the 2e-2 tolerance is far above
    bf16 quantization and it halves the HBM traffic of this DMA-bound kernel.
    """
    import ml_dtypes

    from concourse.bass_utils import run_bass_kernel_spmd

    nc = _get_nc()
    x16 = np.ascontiguousarray(x64.astype(ml_dtypes.bfloat16))
    in_maps = [
        {
            "x": x16[SIGS_PER_CORE * c : SIGS_PER_CORE * (c + 1)],
            "w": w,
        }
        for c in range(N_CORES)
    ]
    res = run_bass_kernel_spmd(
        nc, in_maps, core_ids=list(range(N_CORES)), trace=trace
    )
    out = np.concatenate(
        [np.asarray(res.results[c]["y"]).astype(np.float32) for c in range(N_CORES)],
        axis=0,
    )
    return out, res


def kernel(x, center_freq, q, gain, t=0, **_unused):
    x = np.ascontiguousarray(np.asarray(x), dtype=np.float32)
    assert x.shape == (B_FULL, C_FULL, T_FULL), x.shape
    cf = float(np.asarray(center_freq).reshape(-1)[0])
    qv = float(np.asarray(q).reshape(-1)[0])
    gv = float(np.asarray(gain).reshape(-1)[0])

    h = _impulse_response(cf, qv, gv)
    w = _w_matrix(h)

    x64 = x.reshape(B_FULL * C_FULL, T_FULL)
    out, _ = run_spmd(x64, w, trace=False)
    return out.reshape(B_FULL, C_FULL, T_FULL).astype(np.float32)


# revision 64
# speedup vs baseline: 1.0615x; 1.0615x over previous
"""Biquad peaking-EQ IIR filter on 8 Trainium2 NeuronCores.

Math: the reference applies a 2nd-order IIR (biquad) along time for each of
the 64 independent signals (32 batch x 2 channels, T=524288).  The filter's
poles have magnitude sqrt(a2) ~ 0.919, so the impulse response decays below
~2e-5 after 128 samples.  We compute the zero-state response as a
truncated-FIR convolution (taps 0..g+128 per sample, identical truncation to
the previous kernel, L2 err ~2e-6):

    y[128B + g] = sum_j x[128B + j] T0[j, g] + sum_j x[128(B-1) + j] T1[j, g]

with T0[j,g] = h[g-j] (g>=j), T1[j,g] = h[128+g-j].

Layout trick: tiles are loaded BLOCK-MAJOR straight from DRAM: an SBUF tile
X2[B, j] = x[16384t + 128B + j] is a plain [128, 128-col] strided DMA view
(512-byte contiguous lines), so both the input and the output tiles are
DMA-native and the only on-chip transposes are the PE pass-through
transposes X2 -> X' (X'[j, B]).  X' (cast to bf16 during PSUM evacuation)
is then the MATMUL STATIONARY operand:

    Y2[B, g] = X'[:, 1+128t+B].T @ W0  +  X'[:, 128t+B].T @ W1

where the one-block lookback of the T1 term is just a one-column shift of
the stationary SBUF slice (a zeroed halo column handles the signal start).
The output lands directly in the DMA-native [B, g] orientation - no output
transpose, no strided on-chip copies.

Per core (8 signals): PE = 256 fp32 transposes (2 cyc/row) + 512 bf16
matmuls (1 cyc/row) ~= 131k cycles ~= 55 us; DVE evacuates X' (64x
[128,512] casts), ACT evacuates Y2 (64x [128,512] copies); DMA moves
2 x 16.8 MB at ~360-400 GB/s aggregate => the kernel is DMA-bound.

Sharding: pure data parallel - 64 signals / 8 cores = 8 signals per core.

Scheduling note: every TPB 64-byte instruction has a single semaphore-wait
slot, but Tile's slot-release deps routinely put 2+ waits on one
instruction (walrus then fails with "Too many sync wait commands").
_strip_redundant_waits post-processes the scheduled BIR: it computes
transitive completion guarantees (engine queues are in-order FIFO; an
instruction completes only after its waits held; a semaphore's v-th update
implies its earlier ones) and (a) drops waits provably implied by another
wait on the same instruction, (b) splits any remaining multi-wait set into
single-wait NoOps ahead of the instruction on the same queue.  The patched
BIR is returned via an instance-level to_json_bytes override that
bass2jax's lowering picks up.
"""

import math

import numpy as np

SAMPLE_RATE = 44100.0

# Problem geometry (hardcoded per harness contract).
B_FULL, C_FULL, T_FULL = 32, 2, 524288
N_CORES = 8
SIGS_PER_CORE = (B_FULL * C_FULL) // N_CORES  # 8
L = 128            # block size == PE array dim
NBLK = T_FULL // L          # 4096 blocks per signal
NTILE = NBLK // L           # 32 [128 x 128] tiles per signal
NQ = 4                      # DMA quarters per signal
TPQ = NTILE // NQ           # 8 tiles per quarter


def _filter_coeffs(center_freq: float, q: float, gain: float):
    """torchaudio equalizer_biquad coefficients, normalized by a0 (float64)."""
    g = min(max(gain, 0.1), 10.0)
    w0 = 2.0 * math.pi * center_freq / SAMPLE_RATE
    A = math.exp(g / 40.0 * math.log(10.0))
    alpha = math.sin(w0) / (2.0 * q)
    b0 = 1.0 + alpha * A
    b1 = -2.0 * math.cos(w0)
    b2 = 1.0 - alpha * A
    a0 = 1.0 + alpha / A
    a1 = b1
    a2 = 1.0 - alpha / A
    return b0 / a0, b1 / a0, b2 / a0, a1 / a0, a2 / a0


def _impulse_response(center_freq: float, q: float, gain: float, n: int = 256):
    b0, b1, b2, a1, a2 = _filter_coeffs(center_freq, q, gain)
    h = np.zeros(n, dtype=np.float64)
    x1 = x2 = y1 = y2 = 0.0
    for i in range(n):
        xn = 1.0 if i == 0 else 0.0
        yn = b0 * xn + b1 * x1 + b2 * x2 - a1 * y1 - a2 * y2
        x2, x1 = x1, xn
        y2, y1 = y1, yn
        h[i] = yn
    return h


def _w_matrix(h: np.ndarray):
    """Moving operand W [128, 256] (bf16): cols 0..127 = T0[j,g] = h[g-j]
    (g>=j else 0), cols 128..255 = T1[j,g] = h[128+g-j]."""
    import ml_dtypes

    j = np.arange(L)[:, None]
    g = np.arange(L)[None, :]
    d0 = g - j
    t0 = np.where(d0 >= 0, h[np.clip(d0, 0, len(h) - 1)], 0.0)
    d1 = 128 + g - j  # in [1, 255]
    t1 = h[d1]
    w = np.concatenate([t0, t1], axis=1)
    return w.astype(ml_dtypes.bfloat16)


_NC_CACHE = {}


def _build_nc(n_sigs: int = SIGS_PER_CORE):
    """Build the per-core Bass program (same NEFF on all cores)."""
    import concourse.bass as bass
    import concourse.mybir as mybir
    import concourse.tile as tile
    from concourse.masks import make_identity

    f32 = mybir.dt.float32
    bf16 = mybir.dt.bfloat16
    nc = bass.Bass("TRN2")

    x = nc.dram_tensor("x", [n_sigs, T_FULL], bf16, kind="ExternalInput")
    w = nc.dram_tensor("w", [L, 2 * L], bf16, kind="ExternalInput")
    y = nc.dram_tensor("y", [n_sigs, T_FULL], bf16, kind="ExternalOutput")

    G = 8                 # blocks per output partition line (2 KiB bf16)
    NSUP = NBLK // (G * L)  # 4 supertiles per signal
    # X' source view: (s, B, j) = x[s, 128B + j] -- contiguous 2D per
    # signal, fed to the DMA transpose XBAR ([16, 128] source tiles = 4 KiB
    # sequential DRAM reads).  XBARs all stay on the Sync queue: two
    # concurrently-running XBAR transfers (e.g. from two queues) corrupt
    # each other's output.
    x_r = x[:].rearrange("s (B j) -> s B j", j=L)
    # Paired XBAR source: two DRAM-contiguous signals per transpose, halving
    # the number of (serially dispatched) XBAR instructions on the Sync
    # queue: (c, 8192, 128) covering signals 2c and 2c+1.
    x_p = x[:].rearrange("(c two) (B j) -> c (two B) j", two=2, j=L)
    # Natural G-grouped input view for the PE-transpose signals: partition p
    # line = 2 KiB contiguous per supertile:
    # (s, p, tp, u, j) = x[s, 131072*tp + 1024*p + 128*u + j]
    x_g = x[:].rearrange(
        "s (tp p u j) -> s p tp (u j)", tp=NSUP, p=L, u=G, j=L
    )
    # G-grouped out view: partition m holds blocks {8m+u}, so each partition
    # line is (u, g) = 1024 samples = 2 KiB contiguous DRAM:
    # (s, m, tp, 1024-chunk) = y[s, 131072*tp + 1024*m + chunk]
    y_r = y[:].rearrange("s (tp m ug) -> s m tp ug", tp=NSUP, m=L, ug=G * L)

    with tile.TileContext(nc) as tc:
        with (
            tc.tile_pool(name="consts", bufs=1) as consts,
            tc.tile_pool(name="xp", bufs=4) as xp_pool,
            tc.tile_pool(name="xps", bufs=2) as xps_pool,
            tc.tile_pool(name="xg", bufs=2) as xg_pool,
            tc.tile_pool(name="xt", bufs=6) as xt_pool,
            tc.tile_pool(name="pad", bufs=8) as pad_pool,
            tc.tile_pool(name="yout", bufs=3) as yout_pool,
            tc.tile_pool(name="m_ps", bufs=4, space="PSUM") as m_ps,
            tc.tile_pool(name="t_ps", bufs=3, space="PSUM") as t_ps,
        ):
            w_raw = consts.tile([L, 2 * L], bf16)
            nc.sync.dma_start(w_raw[:], w[:])
            w_sb = consts.tile([L, 2 * L], bf16)
            nc.vector.tensor_copy(w_sb[:], w_raw[:])
            ident = consts.tile([L, L], bf16)
            make_identity(nc, ident[:])

            # Signals on the PE-transpose input path (plain 2 KiB-line DMA +
            # tensor-engine transposes instead of the XBAR).  Measured on
            # hardware: concurrent plain-DMA traffic degrades XBAR packet
            # throughput ~1.5x and its dispatch cost ~1.7x, so the hybrid is
            # a net loss -- keep every signal on the XBAR path.
            PE_SIGS = set()

            # Emit all XBAR loads first: with 8 resident X' tiles they are
            # consecutive on the Sync queue (back-to-back dispatch, input
            # stream never starves) and they claim hardware DMA semaphores
            # before any out-DMA, so the 8-sem round-robin never makes an
            # input wait on an output completion.
            xbar_tiles = {}
            if not PE_SIGS and n_sigs % 2 == 0:
                for c in range(n_sigs // 2):
                    xp2 = xp_pool.tile([L, 2 * NBLK], bf16)
                    nc.sync.dma_start_transpose(xp2[:], x_p[c])
                    xbar_tiles[2 * c] = xp2[:, 0:NBLK]
                    xbar_tiles[2 * c + 1] = xp2[:, NBLK : 2 * NBLK]
            else:
                for s in range(n_sigs):
                    if s not in PE_SIGS:
                        xp = xps_pool.tile([L, NBLK], bf16)
                        nc.sync.dma_start_transpose(xp[:], x_r[s])
                        xbar_tiles[s] = xp[:]

            for s in range(n_sigs):
                y2s = yout_pool.tile([L, NBLK], bf16)
                if s in PE_SIGS:
                    xg = xg_pool.tile([L, NBLK], bf16)
                    nc.scalar.dma_start(
                        xg[:].rearrange("p (tp c) -> p tp c", tp=NSUP),
                        x_g[s],
                    )
                    prev_xt = None
                    for tp in range(NSUP):
                        # X' supertile: col 128u + p <-> block 8p + u.
                        xt = xt_pool.tile([L, G * L], bf16)
                        for g2 in range(2):
                            tps = t_ps.tile([L, 512], bf16, tag="t")
                            for kk in range(4):
                                u = 4 * g2 + kk
                                nc.tensor.transpose(
                                    tps[:, L * kk : L * (kk + 1)],
                                    xg[:, 1024 * tp + L * u : 1024 * tp + L * (u + 1)],
                                    ident[:],
                                )
                            dst = xt[:, 512 * g2 : 512 * (g2 + 1)]
                            if g2 == 0:
                                nc.scalar.copy(dst, tps[:])
                            else:
                                nc.vector.tensor_copy(dst, tps[:])
                        # Pad: col m = block (1024*tp + 8m - 1).
                        pad = pad_pool.tile([L, L], bf16)
                        if tp == 0:
                            nc.vector.memset(pad[:, 0:1], 0.0)
                        else:
                            nc.vector.tensor_copy(
                                pad[:, 0:1], prev_xt[:, G * L - 1 : G * L]
                            )
                        nc.vector.tensor_copy(
                            pad[:, 1:L], xt[:, 7 * L : 7 * L + L - 1]
                        )
                        for half in range(2):
                            ps = m_ps.tile([L, 512], f32, tag="m")
                            for k in range(4):
                                u = 4 * half + k
                                nc.tensor.matmul(
                                    ps[:, L * k : L * (k + 1)],
                                    xt[:, L * u : L * (u + 1)],
                                    w_sb[:, 0:L],
                                    start=True, stop=False,
                                )
                                t1_stat = (
                                    pad[:] if u == 0
                                    else xt[:, L * (u - 1) : L * u]
                                )
                                nc.tensor.matmul(
                                    ps[:, L * k : L * (k + 1)],
                                    t1_stat,
                                    w_sb[:, L : 2 * L],
                                    start=False, stop=True,
                                )
                            base = 1024 * tp + 512 * half
                            dst = y2s[:, base : base + 512]
                            if half == 0:
                                nc.scalar.copy(dst, ps[:])
                            else:
                                nc.vector.tensor_copy(dst, ps[:])
                        prev_xt = xt
                else:
                    # X' via the DMA transpose XBAR (loaded above).  The dst
                    # starts at the tile base (any offset dst produces wrong
                    # XBAR output), so "block -1" halos live in pad tiles.
                    xp = xbar_tiles[s]
                    # Stationary views: (j, tp, u, k) = xp[j, 1024*tp+8k+u]
                    xpv = xp.rearrange(
                        "j (tp k u) -> j tp u k", tp=NSUP, k=L, u=G
                    )
                    for tp in range(NSUP):
                        # Pad tile for the u=0 lookback: col m holds block
                        # (1024*tp + 8m - 1): col 0 = previous supertile's
                        # last block (zero at signal start), cols 1.. = u=7
                        # stride-8 slice shifted by one k.
                        pad = pad_pool.tile([L, L], bf16)
                        if tp == 0:
                            nc.vector.memset(pad[:, 0:1], 0.0)
                        else:
                            nc.vector.tensor_copy(
                                pad[:, 0:1], xp[:, 1024 * tp - 1 : 1024 * tp]
                            )
                        nc.vector.tensor_copy(
                            pad[:, 1:L], xpv[:, tp, G - 1][:, 0 : L - 1]
                        )
                        for half in range(2):  # psum groups of 4 u's
                            ps = m_ps.tile([L, 512], f32, tag="m")
                            for k in range(4):
                                u = 4 * half + k
                                nc.tensor.matmul(
                                    ps[:, L * k : L * (k + 1)],
                                    xpv[:, tp, u],
                                    w_sb[:, 0:L],
                                    start=True, stop=False,
                                )
                                t1_stat = (
                                    pad[:] if u == 0 else xpv[:, tp, u - 1]
                                )
                                nc.tensor.matmul(
                                    ps[:, L * k : L * (k + 1)],
                                    t1_stat,
                                    w_sb[:, L : 2 * L],
                                    start=False, stop=True,
                                )
                            # Evac casts PSUM fp32 -> bf16; split DVE/ACT.
                            base = 1024 * tp + 512 * half
                            dst = y2s[:, base : base + 512]
                            if half == 0:
                                nc.scalar.copy(dst, ps[:])
                            else:
                                nc.vector.tensor_copy(dst, ps[:])
                # One 2KiB-line out-DMA per signal.  Alternate the Scalar and
                # Sync queues: each queue's ring drains one DMA at a time, so
                # two queues double the output stream rate (the Sync queue is
                # idle once the XBARs are dispatched).  Plain-DMA concurrency
                # across queues is safe -- only XBAR-vs-XBAR is not.
                eng = nc.scalar if s % 2 == 0 else nc.sync
                eng.dma_start(
                    y_r[s], y2s[:].rearrange("m (tp ug) -> m tp ug", tp=NSUP)
                )

    return nc


def _strip_redundant_waits(bir_bytes: bytes) -> bytes:
    """PE Matmult/Ldweights lower to TPB instructions with a single
    semaphore-wait slot, but Tile's slot-release deps put 2 waits (old-writer
    PE completion + old-reader DVE completion) on the first toucher of every
    reused PSUM slot.  The PE wait is transitively implied: the DVE evac copy
    whose completion the instruction also waits on had itself waited on those
    PE completions.  Prove the implication with a completion-guarantee
    dataflow (rules: an instruction completes only after its waits hold; TPB
    engine queues are in-order FIFO; a semaphore's v-th update implies its
    earlier updates) and drop provably-redundant waits; raise if a >1-wait
    matmul can't be reduced."""
    import json

    bir = json.loads(bir_bytes)
    insts = []
    containers = []  # (list, index) for each inst, for NoOp insertion

    def walk(block):
        lst = block.get("instructions", [])
        for idx, i in enumerate(lst):
            insts.append(i)
            containers.append((lst, idx))
        for sub in block.get("blocks", []):
            walk(sub)

    for b in bir["functions"][0]["blocks"]:
        walk(b)

    # Per-sem update timeline: list of (cumulative_value, inst_idx).
    timelines = {}
    for k, i in enumerate(insts):
        for u in i.get("sync_info", {}).get("on_update", []) or []:
            if u.get("sync_type") != "semaphore":
                continue
            tl = timelines.setdefault(u["ant_name"], [])
            prev = tl[-1][0] if tl else 0
            tl.append((prev + int(u.get("update_value", 1)), k))

    def producer(sem, val):
        """Index of the instruction whose update first brings sem >= val."""
        tl = timelines.get(sem)
        if not tl:
            return None
        import bisect
        pos = bisect.bisect_left(tl, (val, -1))
        if pos == len(tl):
            return None
        return tl[pos][1]

    IN_ORDER_ENGINES = {"PE", "DVE", "Activation", "Pool", "SP"}
    # complete out-of-band on DMA queues
    NOT_IN_ORDER_OPCODES = {"DMACopy", "DmaTransposeAnt"}

    # guarantees[k]: sem -> max value known to hold when inst k completes.
    guarantees = [dict() for _ in insts]
    prev_by_engine = {}
    preds = []  # per-inst: (same-engine pred, own waits, own updates)
    for k, i in enumerate(insts):
        eng = i.get("engine")
        in_order = eng in IN_ORDER_ENGINES and i.get("opcode") not in NOT_IN_ORDER_OPCODES
        pred = prev_by_engine.get(eng) if in_order else None
        preds.append(pred)
        if in_order:
            prev_by_engine[eng] = k

    def merge(dst, src):
        changed = False
        for s, v in src.items():
            if dst.get(s, 0) < v:
                dst[s] = v
                changed = True
        return changed

    for _pass in range(3):
        changed = False
        for k, i in enumerate(insts):
            g = guarantees[k]
            si = i.get("sync_info", {})
            for w in si.get("on_wait", []) or []:
                if w.get("sync_type") != "semaphore":
                    continue
                v = int(w["wait_value"])
                if g.get(w["ant_name"], 0) < v:
                    g[w["ant_name"]] = v
                    changed = True
                p = producer(w["ant_name"], v)
                if p is not None:
                    changed |= merge(g, guarantees[p])
            # A DMA dispatch may launch before earlier same-queue engine
            # instructions complete, so only non-DMA instructions inherit
            # their queue predecessor's guarantees.
            if preds[k] is not None and i.get("opcode") not in NOT_IN_ORDER_OPCODES:
                changed |= merge(g, guarantees[preds[k]])
        # Own updates fire at completion; same-sem update chains are FIFO
        # (engine queue or DMA queue), so the v-th updater inherits the
        # (v-1)-th updater's guarantees.
        for sem, tl in timelines.items():
            prev_idx = None
            for cum, k in tl:
                if guarantees[k].get(sem, 0) < cum:
                    guarantees[k][sem] = cum
                    changed = True
                if prev_idx is not None:
                    changed |= merge(guarantees[k], guarantees[prev_idx])
                prev_idx = k
        if not changed:
            break

    STRIP_OPCODES = {
        "Matmult", "Ldweights", "TensorCopy", "Memset", "DMACopy",
        "DmaTransposeAnt", "Activation", "TensorScalarAffineSelect",
        "TensorTensor", "TensorScalarPtr", "TensorReduce", "Drain", "NoOp",
    }
    stripped = 0
    inserts = []  # (list, index, [noop dicts])
    for k, i in enumerate(insts):
        if i.get("opcode") not in STRIP_OPCODES:
            continue
        si = i.get("sync_info", {})
        waits = si.get("on_wait", []) or []
        if len(waits) <= 1:
            continue
        # Drop every wait implied by another (not-yet-dropped) wait's
        # producer guarantee.  DMA-class instructions complete out-of-band
        # and their cross-queue ordering is subtle: never REDUCE their
        # waits, only split them onto queue-holding NoOps (always sound).
        kept = list(waits)
        if i.get("opcode") not in NOT_IN_ORDER_OPCODES:
            changed = True
            while changed:
                changed = False
                for w in list(kept):
                    if len(kept) == 1:
                        break
                    for w2 in kept:
                        if w2 is w:
                            continue
                        p = producer(w2["ant_name"], int(w2["wait_value"]))
                        if p is not None and guarantees[p].get(w["ant_name"], 0) >= int(
                            w["wait_value"]
                        ):
                            kept.remove(w)
                            changed = True
                            break
        stripped += len(waits) - len(kept)
        si["on_wait"] = [kept[-1]]
        if len(kept) > 1:
            # Split remaining waits onto single-wait NoOps ahead of the
            # instruction on the same engine queue.
            lst, idx = containers[k]
            noops = [
                {
                    "debug": i.get("debug", 0),
                    "engine": i.get("engine"),
                    "ins": [],
                    "name": f"{i['name']}-w{j}",
                    "opcode": "NoOp",
                    "outs": [],
                    "sync_info": {"on_wait": [w], "on_update": []},
                }
                for j, w in enumerate(kept[:-1])
            ]
            inserts.append((lst, idx, noops))

    # Apply insertions (descending index per list keeps positions valid).
    from collections import defaultdict
    by_list = defaultdict(list)
    for lst, idx, noops in inserts:
        by_list[id(lst)].append((lst, idx, noops))
    for entries in by_list.values():
        for lst, idx, noops in sorted(entries, key=lambda e: -e[1]):
            lst[idx:idx] = noops

    out = json.dumps(bir).encode()
    return out


def audit_waits(bir_bytes):
    """Flag Matmult/Ldweights instructions with more than the single
    hardware wait slot."""
    import json

    bir = json.loads(bir_bytes)
    checked = {
        "Matmult", "Ldweights", "TensorCopy", "Memset", "DMACopy",
        "DmaTransposeAnt", "Activation", "TensorScalarAffineSelect",
        "TensorTensor", "TensorScalarPtr", "TensorReduce",
    }
    bad = []
    def walk(block):
        for i in block.get("instructions", []):
            if i.get("opcode") not in checked:
                continue
            w = i.get("sync_info", {}).get("on_wait", [])
            if len(w) > 1:
                bad.append((i["name"], i.get("opcode"), i.get("engine"),
                            [(x["ant_name"], x["wait_value"]) for x in w]))
        for sub in block.get("blocks", []):
            walk(sub)
    for b in bir["functions"][0]["blocks"]:
        walk(b)
    return bad


def _get_nc(n_sigs: int = SIGS_PER_CORE):
    if n_sigs not in _NC_CACHE:
        nc = _build_nc(n_sigs)
        patched = _strip_redundant_waits(type(nc).to_json_bytes(nc))
        bad = audit_waits(patched)
        if bad:
            raise RuntimeError(f"multi-wait PE instructions remain: {bad[:5]}")
        nc.to_json_bytes = lambda: patched
        _NC_CACHE[n_sigs] = nc
    return _NC_CACHE[n_sigs]


def run_spmd(x64: np.ndarray, w: np.ndarray, trace: bool = False):
    """x64: [64, T] float32, w: [128, 256] bf16 -> [64, T] float32.

    x/y transit DRAM as bf16 (host casts): the 2e-2 tolerance is far above
    bf16 quantization and it halves the HBM traffic of this DMA-bound kernel.
    """
    import ml_dtypes

    from concourse.bass_utils import run_bass_kernel_spmd

    nc = _get_nc()
    x16 = np.ascontiguousarray(x64.astype(ml_dtypes.bfloat16))
    in_maps = [
        {
            "x": x16[SIGS_PER_CORE * c : SIGS_PER_CORE * (c + 1)],
            "w": w,
        }
        for c in range(N_CORES)
    ]
    res = run_bass_kernel_spmd(
        nc, in_maps, core_ids=list(range(N_CORES)), trace=trace
    )
    out = np.concatenate(
        [np.asarray(res.results[c]["y"]).astype(np.float32) for c in range(N_CORES)],
        axis=0,
    )
    return out, res


def kernel(x, center_freq, q, gain, t=0, **_unused):
    x = np.ascontiguousarray(np.asarray(x), dtype=np.float32)
    assert x.shape == (B_FULL, C_FULL, T_FULL), x.shape
    cf = float(np.asarray(center_freq).reshape(-1)[0])
    qv = float(np.asarray(q).reshape(-1)[0])
    gv = float(np.asarray(gain).reshape(-1)[0])

    h = _impulse_response(cf, qv, gv)
    w = _w_matrix(h)

    x64 = x.reshape(B_FULL * C_FULL, T_FULL)
    out, _ = run_spmd(x64, w, trace=False)
    return out.reshape(B_FULL, C_FULL, T_FULL).astype(np.float32)


# revision 65
# speedup vs baseline: 1.0875x; 1.0244x over previous
"""Biquad peaking-EQ IIR filter on 8 Trainium2 NeuronCores.

Math: the reference applies a 2nd-order IIR (biquad) along time for each of
the 64 independent signals (32 batch x 2 channels, T=524288).  The filter's
poles have magnitude sqrt(a2) ~ 0.919, so the impulse response decays below
~2e-5 after 128 samples.  We compute the zero-state response as a
truncated-FIR convolution (taps 0..g+128 per sample, identical truncation to
the previous kernel, L2 err ~2e-6):

    y[128B + g] = sum_j x[128B + j] T0[j, g] + sum_j x[128(B-1) + j] T1[j, g]

with T0[j,g] = h[g-j] (g>=j), T1[j,g] = h[128+g-j].

Layout trick: tiles are loaded BLOCK-MAJOR straight from DRAM: an SBUF tile
X2[B, j] = x[16384t + 128B + j] is a plain [128, 128-col] strided DMA view
(512-byte contiguous lines), so both the input and the output tiles are
DMA-native and the only on-chip transposes are the PE pass-through
transposes X2 -> X' (X'[j, B]).  X' (cast to bf16 during PSUM evacuation)
is then the MATMUL STATIONARY operand:

    Y2[B, g] = X'[:, 1+128t+B].T @ W0  +  X'[:, 128t+B].T @ W1

where the one-block lookback of the T1 term is just a one-column shift of
the stationary SBUF slice (a zeroed halo column handles the signal start).
The output lands directly in the DMA-native [B, g] orientation - no output
transpose, no strided on-chip copies.

Per core (8 signals): PE = 256 fp32 transposes (2 cyc/row) + 512 bf16
matmuls (1 cyc/row) ~= 131k cycles ~= 55 us; DVE evacuates X' (64x
[128,512] casts), ACT evacuates Y2 (64x [128,512] copies); DMA moves
2 x 16.8 MB at ~360-400 GB/s aggregate => the kernel is DMA-bound.

Sharding: pure data parallel - 64 signals / 8 cores = 8 signals per core.

Scheduling note: every TPB 64-byte instruction has a single semaphore-wait
slot, but Tile's slot-release deps routinely put 2+ waits on one
instruction (walrus then fails with "Too many sync wait commands").
_strip_redundant_waits post-processes the scheduled BIR: it computes
transitive completion guarantees (engine queues are in-order FIFO; an
instruction completes only after its waits held; a semaphore's v-th update
implies its earlier ones) and (a) drops waits provably implied by another
wait on the same instruction, (b) splits any remaining multi-wait set into
single-wait NoOps ahead of the instruction on the same queue.  The patched
BIR is returned via an instance-level to_json_bytes override that
bass2jax's lowering picks up.
"""

import math

import numpy as np

SAMPLE_RATE = 44100.0

# Problem geometry (hardcoded per harness contract).
B_FULL, C_FULL, T_FULL = 32, 2, 524288
N_CORES = 8
SIGS_PER_CORE = (B_FULL * C_FULL) // N_CORES  # 8
L = 128            # block size == PE array dim
NBLK = T_FULL // L          # 4096 blocks per signal
NTILE = NBLK // L           # 32 [128 x 128] tiles per signal
NQ = 4                      # DMA quarters per signal
TPQ = NTILE // NQ           # 8 tiles per quarter


def _filter_coeffs(center_freq: float, q: float, gain: float):
    """torchaudio equalizer_biquad coefficients, normalized by a0 (float64)."""
    g = min(max(gain, 0.1), 10.0)
    w0 = 2.0 * math.pi * center_freq / SAMPLE_RATE
    A = math.exp(g / 40.0 * math.log(10.0))
    alpha = math.sin(w0) / (2.0 * q)
    b0 = 1.0 + alpha * A
    b1 = -2.0 * math.cos(w0)
    b2 = 1.0 - alpha * A
    a0 = 1.0 + alpha / A
    a1 = b1
    a2 = 1.0 - alpha / A
    return b0 / a0, b1 / a0, b2 / a0, a1 / a0, a2 / a0


def _impulse_response(center_freq: float, q: float, gain: float, n: int = 256):
    b0, b1, b2, a1, a2 = _filter_coeffs(center_freq, q, gain)
    h = np.zeros(n, dtype=np.float64)
    x1 = x2 = y1 = y2 = 0.0
    for i in range(n):
        xn = 1.0 if i == 0 else 0.0
        yn = b0 * xn + b1 * x1 + b2 * x2 - a1 * y1 - a2 * y2
        x2, x1 = x1, xn
        y2, y1 = y1, yn
        h[i] = yn
    return h


def _w_matrix(h: np.ndarray):
    """Moving operand W [128, 256] (bf16): cols 0..127 = T0[j,g] = h[g-j]
    (g>=j else 0), cols 128..255 = T1[j,g] = h[128+g-j]."""
    import ml_dtypes

    j = np.arange(L)[:, None]
    g = np.arange(L)[None, :]
    d0 = g - j
    t0 = np.where(d0 >= 0, h[np.clip(d0, 0, len(h) - 1)], 0.0)
    d1 = 128 + g - j  # in [1, 255]
    t1 = h[d1]
    w = np.concatenate([t0, t1], axis=1)
    return w.astype(ml_dtypes.bfloat16)


_NC_CACHE = {}


def _build_nc(n_sigs: int = SIGS_PER_CORE):
    """Build the per-core Bass program (same NEFF on all cores)."""
    import concourse.bass as bass
    import concourse.mybir as mybir
    import concourse.tile as tile
    from concourse.masks import make_identity

    f32 = mybir.dt.float32
    bf16 = mybir.dt.bfloat16
    nc = bass.Bass("TRN2")

    x = nc.dram_tensor("x", [n_sigs, T_FULL], bf16, kind="ExternalInput")
    w = nc.dram_tensor("w", [L, 2 * L], bf16, kind="ExternalInput")
    y = nc.dram_tensor("y", [n_sigs, T_FULL], bf16, kind="ExternalOutput")

    G = 8                 # blocks per output partition line (2 KiB bf16)
    NSUP = NBLK // (G * L)  # 4 supertiles per signal
    # X' source view: (s, B, j) = x[s, 128B + j] -- contiguous 2D per
    # signal, fed to the DMA transpose XBAR ([16, 128] source tiles = 4 KiB
    # sequential DRAM reads).  XBARs all stay on the Sync queue: two
    # concurrently-running XBAR transfers (e.g. from two queues) corrupt
    # each other's output.
    x_r = x[:].rearrange("s (B j) -> s B j", j=L)
    # Paired XBAR source: two DRAM-contiguous signals per transpose, halving
    # the number of (serially dispatched) XBAR instructions on the Sync
    # queue: (c, 8192, 128) covering signals 2c and 2c+1.
    x_p = x[:].rearrange("(c two) (B j) -> c (two B) j", two=2, j=L)
    # Natural G-grouped input view for the PE-transpose signals: partition p
    # line = 2 KiB contiguous per supertile:
    # (s, p, tp, u, j) = x[s, 131072*tp + 1024*p + 128*u + j]
    x_g = x[:].rearrange(
        "s (tp p u j) -> s p tp (u j)", tp=NSUP, p=L, u=G, j=L
    )
    # G-grouped out view: partition m holds blocks {8m+u}, so each partition
    # line is (u, g) = 1024 samples = 2 KiB contiguous DRAM:
    # (s, m, tp, 1024-chunk) = y[s, 131072*tp + 1024*m + chunk]
    y_r = y[:].rearrange("s (tp m ug) -> s m tp ug", tp=NSUP, m=L, ug=G * L)

    with tile.TileContext(nc) as tc:
        with (
            tc.tile_pool(name="consts", bufs=1) as consts,
            tc.tile_pool(name="xp", bufs=4) as xp_pool,
            tc.tile_pool(name="xps", bufs=2) as xps_pool,
            tc.tile_pool(name="xg", bufs=2) as xg_pool,
            tc.tile_pool(name="xt", bufs=6) as xt_pool,
            tc.tile_pool(name="pad", bufs=8) as pad_pool,
            tc.tile_pool(name="yout", bufs=3) as yout_pool,
            tc.tile_pool(name="m_ps", bufs=4, space="PSUM") as m_ps,
            tc.tile_pool(name="t_ps", bufs=3, space="PSUM") as t_ps,
        ):
            w_raw = consts.tile([L, 2 * L], bf16)
            nc.sync.dma_start(w_raw[:], w[:])
            w_sb = consts.tile([L, 2 * L], bf16)
            nc.vector.tensor_copy(w_sb[:], w_raw[:])
            ident = consts.tile([L, L], bf16)
            make_identity(nc, ident[:])

            # Signals on the PE-transpose input path (plain 2 KiB-line DMA +
            # tensor-engine transposes instead of the XBAR).  Measured on
            # hardware: concurrent plain-DMA traffic degrades XBAR packet
            # throughput ~1.5x and its dispatch cost ~1.7x, so the hybrid is
            # a net loss -- keep every signal on the XBAR path.
            PE_SIGS = set()

            # Emit all XBAR loads first: with 8 resident X' tiles they are
            # consecutive on the Sync queue (back-to-back dispatch, input
            # stream never starves) and they claim hardware DMA semaphores
            # before any out-DMA, so the 8-sem round-robin never makes an
            # input wait on an output completion.
            xbar_tiles = {}
            if not PE_SIGS and n_sigs % 2 == 0:
                for c in range(n_sigs // 2):
                    xp2 = xp_pool.tile([L, 2 * NBLK], bf16)
                    nc.sync.dma_start_transpose(xp2[:], x_p[c])
                    xbar_tiles[2 * c] = xp2[:, 0:NBLK]
                    xbar_tiles[2 * c + 1] = xp2[:, NBLK : 2 * NBLK]
            else:
                for s in range(n_sigs):
                    if s not in PE_SIGS:
                        xp = xps_pool.tile([L, NBLK], bf16)
                        nc.sync.dma_start_transpose(xp[:], x_r[s])
                        xbar_tiles[s] = xp[:]

            for s in range(n_sigs):
                y2s = yout_pool.tile([L, NBLK], bf16)
                if s in PE_SIGS:
                    xg = xg_pool.tile([L, NBLK], bf16)
                    nc.scalar.dma_start(
                        xg[:].rearrange("p (tp c) -> p tp c", tp=NSUP),
                        x_g[s],
                    )
                    prev_xt = None
                    for tp in range(NSUP):
                        # X' supertile: col 128u + p <-> block 8p + u.
                        xt = xt_pool.tile([L, G * L], bf16)
                        for g2 in range(2):
                            tps = t_ps.tile([L, 512], bf16, tag="t")
                            for kk in range(4):
                                u = 4 * g2 + kk
                                nc.tensor.transpose(
                                    tps[:, L * kk : L * (kk + 1)],
                                    xg[:, 1024 * tp + L * u : 1024 * tp + L * (u + 1)],
                                    ident[:],
                                )
                            dst = xt[:, 512 * g2 : 512 * (g2 + 1)]
                            if g2 == 0:
                                nc.scalar.copy(dst, tps[:])
                            else:
                                nc.vector.tensor_copy(dst, tps[:])
                        # Pad: col m = block (1024*tp + 8m - 1).
                        pad = pad_pool.tile([L, L], bf16)
                        if tp == 0:
                            nc.vector.memset(pad[:, 0:1], 0.0)
                        else:
                            nc.vector.tensor_copy(
                                pad[:, 0:1], prev_xt[:, G * L - 1 : G * L]
                            )
                        nc.vector.tensor_copy(
                            pad[:, 1:L], xt[:, 7 * L : 7 * L + L - 1]
                        )
                        for half in range(2):
                            ps = m_ps.tile([L, 512], f32, tag="m")
                            for k in range(4):
                                u = 4 * half + k
                                nc.tensor.matmul(
                                    ps[:, L * k : L * (k + 1)],
                                    xt[:, L * u : L * (u + 1)],
                                    w_sb[:, 0:L],
                                    start=True, stop=False,
                                )
                                t1_stat = (
                                    pad[:] if u == 0
                                    else xt[:, L * (u - 1) : L * u]
                                )
                                nc.tensor.matmul(
                                    ps[:, L * k : L * (k + 1)],
                                    t1_stat,
                                    w_sb[:, L : 2 * L],
                                    start=False, stop=True,
                                )
                            base = 1024 * tp + 512 * half
                            dst = y2s[:, base : base + 512]
                            if half == 0:
                                nc.scalar.copy(dst, ps[:])
                            else:
                                nc.vector.tensor_copy(dst, ps[:])
                        prev_xt = xt
                else:
                    # X' via the DMA transpose XBAR (loaded above).  The dst
                    # starts at the tile base (any offset dst produces wrong
                    # XBAR output), so "block -1" halos live in pad tiles.
                    xp = xbar_tiles[s]
                    # Stationary views: (j, tp, u, k) = xp[j, 1024*tp+8k+u]
                    xpv = xp.rearrange(
                        "j (tp k u) -> j tp u k", tp=NSUP, k=L, u=G
                    )
                    for tp in range(NSUP):
                        # Pad tile for the u=0 lookback: col m holds block
                        # (1024*tp + 8m - 1): col 0 = previous supertile's
                        # last block (zero at signal start), cols 1.. = u=7
                        # stride-8 slice shifted by one k.
                        # Pads on the (otherwise idle) GpSimd engine so they
                        # never queue behind DVE evacuations -- the u=0
                        # matmul pair of every supertile waits on its pad.
                        pad = pad_pool.tile([L, L], bf16)
                        if tp == 0:
                            nc.gpsimd.memset(pad[:, 0:1], 0.0)
                        else:
                            nc.gpsimd.tensor_copy(
                                pad[:, 0:1], xp[:, 1024 * tp - 1 : 1024 * tp]
                            )
                        nc.gpsimd.tensor_copy(
                            pad[:, 1:L], xpv[:, tp, G - 1][:, 0 : L - 1]
                        )
                        for half in range(2):  # psum groups of 4 u's
                            ps = m_ps.tile([L, 512], f32, tag="m")
                            for k in range(4):
                                u = 4 * half + k
                                nc.tensor.matmul(
                                    ps[:, L * k : L * (k + 1)],
                                    xpv[:, tp, u],
                                    w_sb[:, 0:L],
                                    start=True, stop=False,
                                )
                                t1_stat = (
                                    pad[:] if u == 0 else xpv[:, tp, u - 1]
                                )
                                nc.tensor.matmul(
                                    ps[:, L * k : L * (k + 1)],
                                    t1_stat,
                                    w_sb[:, L : 2 * L],
                                    start=False, stop=True,
                                )
                            # Evac casts PSUM fp32 -> bf16; split DVE/ACT.
                            base = 1024 * tp + 512 * half
                            dst = y2s[:, base : base + 512]
                            if half == 0:
                                nc.scalar.copy(dst, ps[:])
                            else:
                                nc.vector.tensor_copy(dst, ps[:])
                # One 2KiB-line out-DMA per signal.  Alternate the Scalar and
                # Sync queues: each queue's ring drains one DMA at a time, so
                # two queues double the output stream rate (the Sync queue is
                # idle once the XBARs are dispatched).  Plain-DMA concurrency
                # across queues is safe -- only XBAR-vs-XBAR is not.
                eng = nc.scalar if s % 2 == 0 else nc.sync
                eng.dma_start(
                    y_r[s], y2s[:].rearrange("m (tp ug) -> m tp ug", tp=NSUP)
                )

    return nc


def _strip_redundant_waits(bir_bytes: bytes) -> bytes:
    """PE Matmult/Ldweights lower to TPB instructions with a single
    semaphore-wait slot, but Tile's slot-release deps put 2 waits (old-writer
    PE completion + old-reader DVE completion) on the first toucher of every
    reused PSUM slot.  The PE wait is transitively implied: the DVE evac copy
    whose completion the instruction also waits on had itself waited on those
    PE completions.  Prove the implication with a completion-guarantee
    dataflow (rules: an instruction completes only after its waits hold; TPB
    engine queues are in-order FIFO; a semaphore's v-th update implies its
    earlier updates) and drop provably-redundant waits; raise if a >1-wait
    matmul can't be reduced."""
    import json

    bir = json.loads(bir_bytes)
    insts = []
    containers = []  # (list, index) for each inst, for NoOp insertion

    def walk(block):
        lst = block.get("instructions", [])
        for idx, i in enumerate(lst):
            insts.append(i)
            containers.append((lst, idx))
        for sub in block.get("blocks", []):
            walk(sub)

    for b in bir["functions"][0]["blocks"]:
        walk(b)

    # Per-sem update timeline: list of (cumulative_value, inst_idx).
    timelines = {}
    for k, i in enumerate(insts):
        for u in i.get("sync_info", {}).get("on_update", []) or []:
            if u.get("sync_type") != "semaphore":
                continue
            tl = timelines.setdefault(u["ant_name"], [])
            prev = tl[-1][0] if tl else 0
            tl.append((prev + int(u.get("update_value", 1)), k))

    def producer(sem, val):
        """Index of the instruction whose update first brings sem >= val."""
        tl = timelines.get(sem)
        if not tl:
            return None
        import bisect
        pos = bisect.bisect_left(tl, (val, -1))
        if pos == len(tl):
            return None
        return tl[pos][1]

    IN_ORDER_ENGINES = {"PE", "DVE", "Activation", "Pool", "SP"}
    # complete out-of-band on DMA queues
    NOT_IN_ORDER_OPCODES = {"DMACopy", "DmaTransposeAnt"}

    # guarantees[k]: sem -> max value known to hold when inst k completes.
    guarantees = [dict() for _ in insts]
    prev_by_engine = {}
    preds = []  # per-inst: (same-engine pred, own waits, own updates)
    for k, i in enumerate(insts):
        eng = i.get("engine")
        in_order = eng in IN_ORDER_ENGINES and i.get("opcode") not in NOT_IN_ORDER_OPCODES
        pred = prev_by_engine.get(eng) if in_order else None
        preds.append(pred)
        if in_order:
            prev_by_engine[eng] = k

    def merge(dst, src):
        changed = False
        for s, v in src.items():
            if dst.get(s, 0) < v:
                dst[s] = v
                changed = True
        return changed

    for _pass in range(3):
        changed = False
        for k, i in enumerate(insts):
            g = guarantees[k]
            si = i.get("sync_info", {})
            for w in si.get("on_wait", []) or []:
                if w.get("sync_type") != "semaphore":
                    continue
                v = int(w["wait_value"])
                if g.get(w["ant_name"], 0) < v:
                    g[w["ant_name"]] = v
                    changed = True
                p = producer(w["ant_name"], v)
                if p is not None:
                    changed |= merge(g, guarantees[p])
            # A DMA dispatch may launch before earlier same-queue engine
            # instructions complete, so only non-DMA instructions inherit
            # their queue predecessor's guarantees.
            if preds[k] is not None and i.get("opcode") not in NOT_IN_ORDER_OPCODES:
                changed |= merge(g, guarantees[preds[k]])
        # Own updates fire at completion; same-sem update chains are FIFO
        # (engine queue or DMA queue), so the v-th updater inherits the
        # (v-1)-th updater's guarantees.
        for sem, tl in timelines.items():
            prev_idx = None
            for cum, k in tl:
                if guarantees[k].get(sem, 0) < cum:
                    guarantees[k][sem] = cum
                    changed = True
                if prev_idx is not None:
                    changed |= merge(guarantees[k], guarantees[prev_idx])
                prev_idx = k
        if not changed:
            break

    STRIP_OPCODES = {
        "Matmult", "Ldweights", "TensorCopy", "Memset", "DMACopy",
        "DmaTransposeAnt", "Activation", "TensorScalarAffineSelect",
        "TensorTensor", "TensorScalarPtr", "TensorReduce", "Drain", "NoOp",
    }
    stripped = 0
    inserts = []  # (list, index, [noop dicts])
    for k, i in enumerate(insts):
        if i.get("opcode") not in STRIP_OPCODES:
            continue
        si = i.get("sync_info", {})
        waits = si.get("on_wait", []) or []
        if len(waits) <= 1:
            continue
        # Drop every wait implied by another (not-yet-dropped) wait's
        # producer guarantee.  DMA-class instructions complete out-of-band
        # and their cross-queue ordering is subtle: never REDUCE their
        # waits, only split them onto queue-holding NoOps (always sound).
        kept = list(waits)
        if i.get("opcode") not in NOT_IN_ORDER_OPCODES:
            changed = True
            while changed:
                changed = False
                for w in list(kept):
                    if len(kept) == 1:
                        break
                    for w2 in kept:
                        if w2 is w:
                            continue
                        p = producer(w2["ant_name"], int(w2["wait_value"]))
                        if p is not None and guarantees[p].get(w["ant_name"], 0) >= int(
                            w["wait_value"]
                        ):
                            kept.remove(w)
                            changed = True
                            break
        stripped += len(waits) - len(kept)
        si["on_wait"] = [kept[-1]]
        if len(kept) > 1:
            # Split remaining waits onto single-wait NoOps ahead of the
            # instruction on the same engine queue.
            lst, idx = containers[k]
            noops = [
                {
                    "debug": i.get("debug", 0),
                    "engine": i.get("engine"),
                    "ins": [],
                    "name": f"{i['name']}-w{j}",
                    "opcode": "NoOp",
                    "outs": [],
                    "sync_info": {"on_wait": [w], "on_update": []},
                }
                for j, w in enumerate(kept[:-1])
            ]
            inserts.append((lst, idx, noops))

    # Apply insertions (descending index per list keeps positions valid).
    from collections import defaultdict
    by_list = defaultdict(list)
    for lst, idx, noops in inserts:
        by_list[id(lst)].append((lst, idx, noops))
    for entries in by_list.values():
        for lst, idx, noops in sorted(entries, key=lambda e: -e[1]):
            lst[idx:idx] = noops

    out = json.dumps(bir).encode()
    return out


def audit_waits(bir_bytes):
    """Flag Matmult/Ldweights instructions with more than the single
    hardware wait slot."""
    import json

    bir = json.loads(bir_bytes)
    checked = {
        "Matmult", "Ldweights", "TensorCopy", "Memset", "DMACopy",
        "DmaTransposeAnt", "Activation", "TensorScalarAffineSelect",
        "TensorTensor", "TensorScalarPtr", "TensorReduce",
    }
    bad = []
    def walk(block):
        for i in block.get("instructions", []):
            if i.get("opcode") not in checked:
                continue
            w = i.get("sync_info", {}).get("on_wait", [])
            if len(w) > 1:
                bad.append((i["name"], i.get("opcode"), i.get("engine"),
                            [(x["ant_name"], x["wait_value"]) for x in w]))
        for sub in block.get("blocks", []):
            walk(sub)
    for b in bir["functions"][0]["blocks"]:
        walk(b)
    return bad


def _get_nc(n_sigs: int = SIGS_PER_CORE):
    if n_sigs not in _NC_CACHE:
        nc = _build_nc(n_sigs)
        patched = _strip_redundant_waits(type(nc).to_json_bytes(nc))
        bad = audit_waits(patched)
        if bad:
            raise RuntimeError(f"multi-wait PE instructions remain: {bad[:5]}")
        nc.to_json_bytes = lambda: patched
        _NC_CACHE[n_sigs] = nc
    return _NC_CACHE[n_sigs]


def run_spmd(x64: np.ndarray, w: np.ndarray, trace: bool = False):
    """x64: [64, T] float32, w: [128, 256] bf16 -> [64, T] float32.

    x/y transit DRAM as bf16 (host casts): the 2e-2 tolerance is far above
    bf16 quantization and it halves the HBM traffic of this DMA-bound kernel.
    """
    import ml_dtypes

    from concourse.bass_utils import run_bass_kernel_spmd

    nc = _get_nc()
    x16 = np.ascontiguousarray(x64.astype(ml_dtypes.bfloat16))
    in_maps = [
        {
            "x": x16[SIGS_PER_CORE * c : SIGS_PER_CORE * (c + 1)],
            "w": w,
        }
        for c in range(N_CORES)
    ]
    res = run_bass_kernel_spmd(
        nc, in_maps, core_ids=list(range(N_CORES)), trace=trace
    )
    out = np.concatenate(
        [np.asarray(res.results[c]["y"]).astype(np.float32) for c in range(N_CORES)],
        axis=0,
    )
    return out, res


def kernel(x, center_freq, q, gain, t=0, **_unused):
    x = np.ascontiguousarray(np.asarray(x), dtype=np.float32)
    assert x.shape == (B_FULL, C_FULL, T_FULL), x.shape
    cf = float(np.asarray(center_freq).reshape(-1)[0])
    qv = float(np.asarray(q).reshape(-1)[0])
    gv = float(np.asarray(gain).reshape(-1)[0])

    h = _impulse_response(cf, qv, gv)
    w = _w_matrix(h)

    x64 = x.reshape(B_FULL * C_FULL, T_FULL)
    out, _ = run_spmd(x64, w, trace=False)
    return out.reshape(B_FULL, C_FULL, T_FULL).astype(np.float32)


# revision 66
# speedup vs baseline: 1.1265x; 1.0359x over previous
"""Biquad peaking-EQ IIR filter on 8 Trainium2 NeuronCores.

Math: the reference applies a 2nd-order IIR (biquad) along time for each of
the 64 independent signals (32 batch x 2 channels, T=524288).  The filter's
poles have magnitude sqrt(a2) ~ 0.919, so the impulse response decays below
~2e-5 after 128 samples.  We compute the zero-state response as a
truncated-FIR convolution (taps 0..g+128 per sample, identical truncation to
the previous kernel, L2 err ~2e-6):

    y[128B + g] = sum_j x[128B + j] T0[j, g] + sum_j x[128(B-1) + j] T1[j, g]

with T0[j,g] = h[g-j] (g>=j), T1[j,g] = h[128+g-j].

Layout trick: tiles are loaded BLOCK-MAJOR straight from DRAM: an SBUF tile
X2[B, j] = x[16384t + 128B + j] is a plain [128, 128-col] strided DMA view
(512-byte contiguous lines), so both the input and the output tiles are
DMA-native and the only on-chip transposes are the PE pass-through
transposes X2 -> X' (X'[j, B]).  X' (cast to bf16 during PSUM evacuation)
is then the MATMUL STATIONARY operand:

    Y2[B, g] = X'[:, 1+128t+B].T @ W0  +  X'[:, 128t+B].T @ W1

where the one-block lookback of the T1 term is just a one-column shift of
the stationary SBUF slice (a zeroed halo column handles the signal start).
The output lands directly in the DMA-native [B, g] orientation - no output
transpose, no strided on-chip copies.

Per core (8 signals): PE = 256 fp32 transposes (2 cyc/row) + 512 bf16
matmuls (1 cyc/row) ~= 131k cycles ~= 55 us; DVE evacuates X' (64x
[128,512] casts), ACT evacuates Y2 (64x [128,512] copies); DMA moves
2 x 16.8 MB at ~360-400 GB/s aggregate => the kernel is DMA-bound.

Sharding: pure data parallel - 64 signals / 8 cores = 8 signals per core.

Scheduling note: every TPB 64-byte instruction has a single semaphore-wait
slot, but Tile's slot-release deps routinely put 2+ waits on one
instruction (walrus then fails with "Too many sync wait commands").
_strip_redundant_waits post-processes the scheduled BIR: it computes
transitive completion guarantees (engine queues are in-order FIFO; an
instruction completes only after its waits held; a semaphore's v-th update
implies its earlier ones) and (a) drops waits provably implied by another
wait on the same instruction, (b) splits any remaining multi-wait set into
single-wait NoOps ahead of the instruction on the same queue.  The patched
BIR is returned via an instance-level to_json_bytes override that
bass2jax's lowering picks up.
"""

import math

import numpy as np

SAMPLE_RATE = 44100.0

# Problem geometry (hardcoded per harness contract).
B_FULL, C_FULL, T_FULL = 32, 2, 524288
N_CORES = 8
SIGS_PER_CORE = (B_FULL * C_FULL) // N_CORES  # 8
L = 128            # block size == PE array dim
NBLK = T_FULL // L          # 4096 blocks per signal
NTILE = NBLK // L           # 32 [128 x 128] tiles per signal
NQ = 4                      # DMA quarters per signal
TPQ = NTILE // NQ           # 8 tiles per quarter


def _filter_coeffs(center_freq: float, q: float, gain: float):
    """torchaudio equalizer_biquad coefficients, normalized by a0 (float64)."""
    g = min(max(gain, 0.1), 10.0)
    w0 = 2.0 * math.pi * center_freq / SAMPLE_RATE
    A = math.exp(g / 40.0 * math.log(10.0))
    alpha = math.sin(w0) / (2.0 * q)
    b0 = 1.0 + alpha * A
    b1 = -2.0 * math.cos(w0)
    b2 = 1.0 - alpha * A
    a0 = 1.0 + alpha / A
    a1 = b1
    a2 = 1.0 - alpha / A
    return b0 / a0, b1 / a0, b2 / a0, a1 / a0, a2 / a0


def _impulse_response(center_freq: float, q: float, gain: float, n: int = 256):
    b0, b1, b2, a1, a2 = _filter_coeffs(center_freq, q, gain)
    h = np.zeros(n, dtype=np.float64)
    x1 = x2 = y1 = y2 = 0.0
    for i in range(n):
        xn = 1.0 if i == 0 else 0.0
        yn = b0 * xn + b1 * x1 + b2 * x2 - a1 * y1 - a2 * y2
        x2, x1 = x1, xn
        y2, y1 = y1, yn
        h[i] = yn
    return h


def _w_matrix(h: np.ndarray):
    """Moving operand W [128, 256] (bf16): cols 0..127 = T0[j,g] = h[g-j]
    (g>=j else 0), cols 128..255 = T1[j,g] = h[128+g-j]."""
    import ml_dtypes

    j = np.arange(L)[:, None]
    g = np.arange(L)[None, :]
    d0 = g - j
    t0 = np.where(d0 >= 0, h[np.clip(d0, 0, len(h) - 1)], 0.0)
    d1 = 128 + g - j  # in [1, 255]
    t1 = h[d1]
    w = np.concatenate([t0, t1], axis=1)
    return w.astype(ml_dtypes.bfloat16)


_NC_CACHE = {}


def _build_nc(n_sigs: int = SIGS_PER_CORE):
    """Build the per-core Bass program (same NEFF on all cores)."""
    import concourse.bass as bass
    import concourse.mybir as mybir
    import concourse.tile as tile
    from concourse.masks import make_identity

    f32 = mybir.dt.float32
    bf16 = mybir.dt.bfloat16
    nc = bass.Bass("TRN2")

    x = nc.dram_tensor("x", [n_sigs, T_FULL], bf16, kind="ExternalInput")
    w = nc.dram_tensor("w", [L, 2 * L], bf16, kind="ExternalInput")
    y = nc.dram_tensor("y", [n_sigs, T_FULL], bf16, kind="ExternalOutput")

    G = 8                 # blocks per output partition line (2 KiB bf16)
    NSUP = NBLK // (G * L)  # 4 supertiles per signal
    # X' source view: (s, B, j) = x[s, 128B + j] -- contiguous 2D per
    # signal, fed to the DMA transpose XBAR ([16, 128] source tiles = 4 KiB
    # sequential DRAM reads).  XBARs all stay on the Sync queue: two
    # concurrently-running XBAR transfers (e.g. from two queues) corrupt
    # each other's output.
    x_r = x[:].rearrange("s (B j) -> s B j", j=L)
    # Paired XBAR source: two DRAM-contiguous signals per transpose, halving
    # the number of (serially dispatched) XBAR instructions on the Sync
    # queue: (c, 8192, 128) covering signals 2c and 2c+1.
    x_p = x[:].rearrange("(c two) (B j) -> c (two B) j", two=2, j=L)
    # Natural G-grouped input view for the PE-transpose signals: partition p
    # line = 2 KiB contiguous per supertile:
    # (s, p, tp, u, j) = x[s, 131072*tp + 1024*p + 128*u + j]
    x_g = x[:].rearrange(
        "s (tp p u j) -> s p tp (u j)", tp=NSUP, p=L, u=G, j=L
    )
    # G-grouped out view: partition m holds blocks {8m+u}, so each partition
    # line is (u, g) = 1024 samples = 2 KiB contiguous DRAM:
    # (s, m, tp, 1024-chunk) = y[s, 131072*tp + 1024*m + chunk]
    y_r = y[:].rearrange("s (tp m ug) -> s m tp ug", tp=NSUP, m=L, ug=G * L)

    with tile.TileContext(nc) as tc:
        with (
            tc.tile_pool(name="consts", bufs=1) as consts,
            tc.tile_pool(name="xp", bufs=4) as xp_pool,
            tc.tile_pool(name="xps", bufs=2) as xps_pool,
            tc.tile_pool(name="xg", bufs=2) as xg_pool,
            tc.tile_pool(name="xt", bufs=6) as xt_pool,
            tc.tile_pool(name="pad", bufs=8) as pad_pool,
            tc.tile_pool(name="yout", bufs=3) as yout_pool,
            tc.tile_pool(name="m_ps", bufs=4, space="PSUM") as m_ps,
            tc.tile_pool(name="t_ps", bufs=3, space="PSUM") as t_ps,
        ):
            w_raw = consts.tile([L, 2 * L], bf16)
            nc.sync.dma_start(w_raw[:], w[:])
            w_sb = consts.tile([L, 2 * L], bf16)
            nc.vector.tensor_copy(w_sb[:], w_raw[:])
            ident = consts.tile([L, L], bf16)
            make_identity(nc, ident[:])

            # Signals on the PE-transpose input path (plain 2 KiB-line DMA +
            # tensor-engine transposes instead of the XBAR).  Measured on
            # hardware: concurrent plain-DMA traffic degrades XBAR packet
            # throughput ~1.5x and its dispatch cost ~1.7x, so the hybrid is
            # a net loss -- keep every signal on the XBAR path.
            PE_SIGS = set()

            # Emit all XBAR loads first: with 8 resident X' tiles they are
            # consecutive on the Sync queue (back-to-back dispatch, input
            # stream never starves) and they claim hardware DMA semaphores
            # before any out-DMA, so the 8-sem round-robin never makes an
            # input wait on an output completion.
            xbar_tiles = {}
            if not PE_SIGS and n_sigs % 2 == 0:
                for c in range(n_sigs // 2):
                    xp2 = xp_pool.tile([L, 2 * NBLK], bf16)
                    nc.sync.dma_start_transpose(xp2[:], x_p[c])
                    xbar_tiles[2 * c] = xp2[:, 0:NBLK]
                    xbar_tiles[2 * c + 1] = xp2[:, NBLK : 2 * NBLK]
            else:
                for s in range(n_sigs):
                    if s not in PE_SIGS:
                        xp = xps_pool.tile([L, NBLK], bf16)
                        nc.sync.dma_start_transpose(xp[:], x_r[s])
                        xbar_tiles[s] = xp[:]

            for s in range(n_sigs):
                y2s = yout_pool.tile([L, NBLK], bf16)
                if s in PE_SIGS:
                    xg = xg_pool.tile([L, NBLK], bf16)
                    nc.scalar.dma_start(
                        xg[:].rearrange("p (tp c) -> p tp c", tp=NSUP),
                        x_g[s],
                    )
                    prev_xt = None
                    for tp in range(NSUP):
                        # X' supertile: col 128u + p <-> block 8p + u.
                        xt = xt_pool.tile([L, G * L], bf16)
                        for g2 in range(2):
                            tps = t_ps.tile([L, 512], bf16, tag="t")
                            for kk in range(4):
                                u = 4 * g2 + kk
                                nc.tensor.transpose(
                                    tps[:, L * kk : L * (kk + 1)],
                                    xg[:, 1024 * tp + L * u : 1024 * tp + L * (u + 1)],
                                    ident[:],
                                )
                            dst = xt[:, 512 * g2 : 512 * (g2 + 1)]
                            if g2 == 0:
                                nc.scalar.copy(dst, tps[:])
                            else:
                                nc.vector.tensor_copy(dst, tps[:])
                        # Pad: col m = block (1024*tp + 8m - 1).
                        pad = pad_pool.tile([L, L], bf16)
                        if tp == 0:
                            nc.vector.memset(pad[:, 0:1], 0.0)
                        else:
                            nc.vector.tensor_copy(
                                pad[:, 0:1], prev_xt[:, G * L - 1 : G * L]
                            )
                        nc.vector.tensor_copy(
                            pad[:, 1:L], xt[:, 7 * L : 7 * L + L - 1]
                        )
                        for half in range(2):
                            ps = m_ps.tile([L, 512], f32, tag="m")
                            for k in range(4):
                                u = 4 * half + k
                                nc.tensor.matmul(
                                    ps[:, L * k : L * (k + 1)],
                                    xt[:, L * u : L * (u + 1)],
                                    w_sb[:, 0:L],
                                    start=True, stop=False,
                                )
                                t1_stat = (
                                    pad[:] if u == 0
                                    else xt[:, L * (u - 1) : L * u]
                                )
                                nc.tensor.matmul(
                                    ps[:, L * k : L * (k + 1)],
                                    t1_stat,
                                    w_sb[:, L : 2 * L],
                                    start=False, stop=True,
                                )
                            base = 1024 * tp + 512 * half
                            dst = y2s[:, base : base + 512]
                            if half == 0:
                                nc.scalar.copy(dst, ps[:])
                            else:
                                nc.vector.tensor_copy(dst, ps[:])
                        prev_xt = xt
                else:
                    # X' via the DMA transpose XBAR (loaded above).  The dst
                    # starts at the tile base (any offset dst produces wrong
                    # XBAR output), so "block -1" halos live in pad tiles.
                    xp = xbar_tiles[s]
                    # Stationary views: (j, tp, u, k) = xp[j, 1024*tp+8k+u]
                    xpv = xp.rearrange(
                        "j (tp k u) -> j tp u k", tp=NSUP, k=L, u=G
                    )
                    for tp in range(NSUP):
                        # Pad tile for the u=0 lookback: col m holds block
                        # (1024*tp + 8m - 1): col 0 = previous supertile's
                        # last block (zero at signal start), cols 1.. = u=7
                        # stride-8 slice shifted by one k.
                        pad = pad_pool.tile([L, L], bf16)
                        if tp == 0:
                            nc.vector.memset(pad[:, 0:1], 0.0)
                        else:
                            nc.vector.tensor_copy(
                                pad[:, 0:1], xp[:, 1024 * tp - 1 : 1024 * tp]
                            )
                        nc.vector.tensor_copy(
                            pad[:, 1:L], xpv[:, tp, G - 1][:, 0 : L - 1]
                        )
                        for half in range(2):  # psum groups of 4 u's
                            ps = m_ps.tile([L, 512], f32, tag="m")
                            for k in range(4):
                                u = 4 * half + k
                                nc.tensor.matmul(
                                    ps[:, L * k : L * (k + 1)],
                                    xpv[:, tp, u],
                                    w_sb[:, 0:L],
                                    start=True, stop=False,
                                )
                                t1_stat = (
                                    pad[:] if u == 0 else xpv[:, tp, u - 1]
                                )
                                nc.tensor.matmul(
                                    ps[:, L * k : L * (k + 1)],
                                    t1_stat,
                                    w_sb[:, L : 2 * L],
                                    start=False, stop=True,
                                )
                            # Evac casts PSUM fp32 -> bf16; split DVE/ACT.
                            base = 1024 * tp + 512 * half
                            dst = y2s[:, base : base + 512]
                            if half == 0:
                                nc.scalar.copy(dst, ps[:])
                            else:
                                nc.vector.tensor_copy(dst, ps[:])
                # One 2KiB-line out-DMA per signal.  Alternate the Scalar and
                # Sync queues: each queue's ring drains one DMA at a time, so
                # two queues double the output stream rate (the Sync queue is
                # idle once the XBARs are dispatched).  Plain-DMA concurrency
                # across queues is safe -- only XBAR-vs-XBAR is not.
                eng = nc.scalar if s % 2 == 0 else nc.sync
                eng.dma_start(
                    y_r[s], y2s[:].rearrange("m (tp ug) -> m tp ug", tp=NSUP)
                )

    return nc


def _strip_redundant_waits(bir_bytes: bytes) -> bytes:
    """PE Matmult/Ldweights lower to TPB instructions with a single
    semaphore-wait slot, but Tile's slot-release deps put 2 waits (old-writer
    PE completion + old-reader DVE completion) on the first toucher of every
    reused PSUM slot.  The PE wait is transitively implied: the DVE evac copy
    whose completion the instruction also waits on had itself waited on those
    PE completions.  Prove the implication with a completion-guarantee
    dataflow (rules: an instruction completes only after its waits hold; TPB
    engine queues are in-order FIFO; a semaphore's v-th update implies its
    earlier updates) and drop provably-redundant waits; raise if a >1-wait
    matmul can't be reduced."""
    import json

    bir = json.loads(bir_bytes)
    insts = []
    containers = []  # (list, index) for each inst, for NoOp insertion

    def walk(block):
        lst = block.get("instructions", [])
        for idx, i in enumerate(lst):
            insts.append(i)
            containers.append((lst, idx))
        for sub in block.get("blocks", []):
            walk(sub)

    for b in bir["functions"][0]["blocks"]:
        walk(b)

    # Per-sem update timeline: list of (cumulative_value, inst_idx).
    timelines = {}
    for k, i in enumerate(insts):
        for u in i.get("sync_info", {}).get("on_update", []) or []:
            if u.get("sync_type") != "semaphore":
                continue
            tl = timelines.setdefault(u["ant_name"], [])
            prev = tl[-1][0] if tl else 0
            tl.append((prev + int(u.get("update_value", 1)), k))

    def producer(sem, val):
        """Index of the instruction whose update first brings sem >= val."""
        tl = timelines.get(sem)
        if not tl:
            return None
        import bisect
        pos = bisect.bisect_left(tl, (val, -1))
        if pos == len(tl):
            return None
        return tl[pos][1]

    IN_ORDER_ENGINES = {"PE", "DVE", "Activation", "Pool", "SP"}
    # complete out-of-band on DMA queues
    NOT_IN_ORDER_OPCODES = {"DMACopy", "DmaTransposeAnt"}

    # guarantees[k]: sem -> max value known to hold when inst k completes.
    guarantees = [dict() for _ in insts]
    prev_by_engine = {}
    preds = []  # per-inst: (same-engine pred, own waits, own updates)
    for k, i in enumerate(insts):
        eng = i.get("engine")
        in_order = eng in IN_ORDER_ENGINES and i.get("opcode") not in NOT_IN_ORDER_OPCODES
        pred = prev_by_engine.get(eng) if in_order else None
        preds.append(pred)
        if in_order:
            prev_by_engine[eng] = k

    def merge(dst, src):
        changed = False
        for s, v in src.items():
            if dst.get(s, 0) < v:
                dst[s] = v
                changed = True
        return changed

    for _pass in range(3):
        changed = False
        for k, i in enumerate(insts):
            g = guarantees[k]
            si = i.get("sync_info", {})
            for w in si.get("on_wait", []) or []:
                if w.get("sync_type") != "semaphore":
                    continue
                v = int(w["wait_value"])
                if g.get(w["ant_name"], 0) < v:
                    g[w["ant_name"]] = v
                    changed = True
                p = producer(w["ant_name"], v)
                if p is not None:
                    changed |= merge(g, guarantees[p])
            # A DMA dispatch may launch before earlier same-queue engine
            # instructions complete, so only non-DMA instructions inherit
            # their queue predecessor's guarantees.
            if preds[k] is not None and i.get("opcode") not in NOT_IN_ORDER_OPCODES:
                changed |= merge(g, guarantees[preds[k]])
        # Own updates fire at completion; same-sem update chains are FIFO
        # (engine queue or DMA queue), so the v-th updater inherits the
        # (v-1)-th updater's guarantees.
        for sem, tl in timelines.items():
            prev_idx = None
            for cum, k in tl:
                if guarantees[k].get(sem, 0) < cum:
                    guarantees[k][sem] = cum
                    changed = True
                if prev_idx is not None:
                    changed |= merge(guarantees[k], guarantees[prev_idx])
                prev_idx = k
        if not changed:
            break

    STRIP_OPCODES = {
        "Matmult", "Ldweights", "TensorCopy", "Memset", "DMACopy",
        "DmaTransposeAnt", "Activation", "TensorScalarAffineSelect",
        "TensorTensor", "TensorScalarPtr", "TensorReduce", "Drain", "NoOp",
    }
    stripped = 0
    inserts = []  # (list, index, [noop dicts])
    for k, i in enumerate(insts):
        if i.get("opcode") not in STRIP_OPCODES:
            continue
        si = i.get("sync_info", {})
        waits = si.get("on_wait", []) or []
        if len(waits) <= 1:
            continue
        # Drop every wait implied by another (not-yet-dropped) wait's
        # producer guarantee.  DMA-class instructions complete out-of-band
        # and their cross-queue ordering is subtle: never REDUCE their
        # waits, only split them onto queue-holding NoOps (always sound).
        kept = list(waits)
        if i.get("opcode") not in NOT_IN_ORDER_OPCODES:
            changed = True
            while changed:
                changed = False
                for w in list(kept):
                    if len(kept) == 1:
                        break
                    for w2 in kept:
                        if w2 is w:
                            continue
                        p = producer(w2["ant_name"], int(w2["wait_value"]))
                        if p is not None and guarantees[p].get(w["ant_name"], 0) >= int(
                            w["wait_value"]
                        ):
                            kept.remove(w)
                            changed = True
                            break
        stripped += len(waits) - len(kept)
        si["on_wait"] = [kept[-1]]
        if len(kept) > 1:
            # Split remaining waits onto single-wait NoOps ahead of the
            # instruction on the same engine queue.
            lst, idx = containers[k]
            noops = [
                {
                    "debug": i.get("debug", 0),
                    "engine": i.get("engine"),
                    "ins": [],
                    "name": f"{i['name']}-w{j}",
                    "opcode": "NoOp",
                    "outs": [],
                    "sync_info": {"on_wait": [w], "on_update": []},
                }
                for j, w in enumerate(kept[:-1])
            ]
            inserts.append((lst, idx, noops))

    # Apply insertions (descending index per list keeps positions valid).
    from collections import defaultdict
    by_list = defaultdict(list)
    for lst, idx, noops in inserts:
        by_list[id(lst)].append((lst, idx, noops))
    for entries in by_list.values():
        for lst, idx, noops in sorted(entries, key=lambda e: -e[1]):
            lst[idx:idx] = noops

    out = json.dumps(bir).encode()
    return out


def audit_waits(bir_bytes):
    """Flag Matmult/Ldweights instructions with more than the single
    hardware wait slot."""
    import json

    bir = json.loads(bir_bytes)
    checked = {
        "Matmult", "Ldweights", "TensorCopy", "Memset", "DMACopy",
        "DmaTransposeAnt", "Activation", "TensorScalarAffineSelect",
        "TensorTensor", "TensorScalarPtr", "TensorReduce",
    }
    bad = []
    def walk(block):
        for i in block.get("instructions", []):
            if i.get("opcode") not in checked:
                continue
            w = i.get("sync_info", {}).get("on_wait", [])
            if len(w) > 1:
                bad.append((i["name"], i.get("opcode"), i.get("engine"),
                            [(x["ant_name"], x["wait_value"]) for x in w]))
        for sub in block.get("blocks", []):
            walk(sub)
    for b in bir["functions"][0]["blocks"]:
        walk(b)
    return bad


def _get_nc(n_sigs: int = SIGS_PER_CORE):
    if n_sigs not in _NC_CACHE:
        nc = _build_nc(n_sigs)
        patched = _strip_redundant_waits(type(nc).to_json_bytes(nc))
        bad = audit_waits(patched)
        if bad:
            raise RuntimeError(f"multi-wait PE instructions remain: {bad[:5]}")
        nc.to_json_bytes = lambda: patched
        _NC_CACHE[n_sigs] = nc
    return _NC_CACHE[n_sigs]


def run_spmd(x64: np.ndarray, w: np.ndarray, trace: bool = False):
    """x64: [64, T] float32, w: [128, 256] bf16 -> [64, T] float32.

    x/y transit DRAM as bf16 (host casts): the 2e-2 tolerance is far above
    bf16 quantization and it halves the HBM traffic of this DMA-bound kernel.
    """
    import ml_dtypes

    from concourse.bass_utils import run_bass_kernel_spmd

    nc = _get_nc()
    x16 = np.ascontiguousarray(x64.astype(ml_dtypes.bfloat16))
    in_maps = [
        {
            "x": x16[SIGS_PER_CORE * c : SIGS_PER_CORE * (c + 1)],
            "w": w,
        }
        for c in range(N_CORES)
    ]
    res = run_bass_kernel_spmd(
        nc, in_maps, core_ids=list(range(N_CORES)), trace=trace
    )
    out = np.concatenate(
        [np.asarray(res.results[c]["y"]).astype(np.float32) for c in range(N_CORES)],
        axis=0,
    )
    return out, res


def kernel(x, center_freq, q, gain, t=0, **_unused):
    x = np.ascontiguousarray(np.asarray(x), dtype=np.float32)
    assert x.shape == (B_FULL, C_FULL, T_FULL), x.shape
    cf = float(np.asarray(center_freq).reshape(-1)[0])
    qv = float(np.asarray(q).reshape(-1)[0])
    gv = float(np.asarray(gain).reshape(-1)[0])

    h = _impulse_response(cf, qv, gv)
    w = _w_matrix(h)

    x64 = x.reshape(B_FULL * C_FULL, T_FULL)
    out, _ = run_spmd(x64, w, trace=False)
    return out.reshape(B_FULL, C_FULL, T_FULL).astype(np.float32)
